# revision 40
# baseline (speedup 1.0000x reference)
"""Child-Sum TreeLSTM over a complete 8-ary tree (depth 6, 299593 nodes) on
8 Trainium2 NeuronCores.

Sharding: the 8 independent subtrees under the root go one-per-core; each core
runs the bottom-up sweep for levels L6 (leaves) .. L4 of its subtree and ships
the L4 partials (hs4 = sum of child h, fc4 = sum of f*c: 512 nodes) back.
The L4 iou/cell update plus the tiny latency-bound tip of the tree (L3/L2/L1
per subtree + the root) run vectorized in fp32 on the host — everything
level-5-down (99.8% of nodes, all 299,592 child-edge forget gates and child
sums) stays on the device.  This removes a ~25us serial small-op tail from
the device critical path and improves accuracy (fp32 tops).

Device layout: feature-major ([128 features on partitions, nodes on free
dim]), fp16 everywhere, matmuls accumulate in fp32 PSUM.  The kernel is
ScalarE(ACT)-bound: every sigmoid/tanh costs 1 column/cycle at 1.2 GHz and
the model needs 4 activations per node + 1 per child edge.  Two mitigations:
  - the leaf tanh(c) (the only activation whose argument is provably in
    [-1, 1]: c_leaf = sigmoid(i)*tanh(u)) is split: the first SPLIT_TCT
    columns per chunk run on ACT, the rest as a quintic odd polynomial on the
    (less loaded) vector engine;
  - elementwise DVE ops are batched to 4096-wide ops to amortize per-op
    overhead, and the per-child forget path accumulates W_fh@h_child and
    W_fx@x_parent into one PSUM via a broadcast-rhs matmul (PE has slack).
"""

import os

import numpy as np

import concourse.tile as tile
from concourse import bacc, mybir
from concourse.bass_utils import run_bass_kernel_spmd

F16 = mybir.dt.float16
F32 = mybir.dt.float32
SIG = mybir.ActivationFunctionType.Sigmoid
TANH = mybir.ActivationFunctionType.Tanh
MUL = mybir.AluOpType.mult
ADD = mybir.AluOpType.add

BRANCH = 8
DEPTH = 6
MEM = 128
IN_DIM = 128
N_NODES = (BRANCH ** (DEPTH + 1) - 1) // (BRANCH - 1)  # 299593

# Per-subtree (local) levels L1..L6: sizes 8^(L-1), offsets into the per-core
# x tensor (levels concatenated in order L1..L6).
LVL_SIZES = [BRANCH**i for i in range(DEPTH)]  # [1, 8, 64, 512, 4096, 32768]
LVL_OFF = [sum(LVL_SIZES[:i]) for i in range(DEPTH)]  # [0,1,9,73,585,4681]
SUB_N = sum(LVL_SIZES)  # 37449

LEAF_OFF = LVL_OFF[5]  # 4681
L5_OFF = LVL_OFF[4]  # 585
L4_OFF = LVL_OFF[3]  # 73
N_CHUNKS = 8  # leaf/L5 fused chunks: 512 L5-parents (4096 leaves) each
# chunks whose SECOND half also uses the DVE quintic (ACT<->DVE balance knob)
QUINTIC_BOTH = ()

# Leaf tanh(c) split: columns [0:SPLIT_TCT) per 4096-chunk on ACT, the rest
# via the DVE quintic tanh(x) ~ x*(1 + A5 x^2 + B5 x^4) (max err 2.1e-3 on
# [-1,1]; |c_leaf| < 1 always since c_leaf = sigmoid*tanh).
SPLIT_TCT = 2048
A5 = -0.32159217
B5 = 0.08530049

LAST_RESULTS = None  # stash for test harness introspection


def _tree_reduce8(nc, pool, src3, m, dst, tag):
    """dst[128, m] = sum over last axis of src3 [128, m, 8] (fp16 tree adds)."""
    t1 = pool.tile([128, m * 4], F16, tag=tag + "_t1")
    t1v = t1[:].rearrange("p (m f) -> p m f", f=4)
    nc.vector.tensor_add(t1v, src3[:, :, 0:4], src3[:, :, 4:8])
    t2 = pool.tile([128, m * 2], F16, tag=tag + "_t2")
    t2v = t2[:].rearrange("p (m f) -> p m f", f=2)
    nc.vector.tensor_add(t2v, t1v[:, :, 0:2], t1v[:, :, 2:4])
    dstv = dst.rearrange("p (m f) -> p m f", f=1)
    nc.vector.tensor_add(dstv, t2v[:, :, 0:1], t2v[:, :, 1:2])


def _gate_group(nc, psum, gg, gf, W, m, c_ch, h_ch, x_par, hs_dst, fc_dst):
    """Forget gates + child sums for m parents (8m children).

    c_ch/h_ch: [128, 8m] fp16 child states; x_par: [128, m] fp16 parent x.
    Writes hs_dst/fc_dst [128, m] (h child-sum, sum of f*c).
    DVE ops run at full 8m width; PE/ACT work in <=2048-col sub-pieces
    (PSUM bank limit).
    """
    cols = 8 * m
    # h child-sum first: it doesn't depend on the forget sigmoid, so the
    # DVE computes it while ACT runs f, and it feeds the next level's
    # (critical) iou_h matmuls earlier.
    hch3 = h_ch.rearrange("p (m f) -> p m f", f=8)
    _tree_reduce8(nc, gg, hch3, m, hs_dst, "hs")
    f = gf.tile([128, cols], F16, tag="f")
    for s0 in range(0, cols, 2048):
        e0 = min(cols, s0 + 2048)
        pf = psum.tile([128, e0 - s0], F32, tag="pg", name=f"pf_{s0}")
        # weight-major order: all wfh matmuls, then all wfx, so LDWEIGHTS is
        # not re-issued per matmul.
        for s in range(s0, e0, 512):
            e = min(e0, s + 512)
            nc.tensor.matmul(
                pf[:, s - s0 : e - s0], W["wfh"][:], h_ch[:, s:e],
                start=True, stop=False,
            )
        for s in range(s0, e0, 512):
            e = min(e0, s + 512)
            xb = (
                x_par[:, s // 8 : e // 8]
                .rearrange("p (m o) -> p m o", o=1)
                .broadcast_to([128, (e - s) // 8, 8])
            )
            nc.tensor.matmul(
                pf[:, s - s0 : e - s0], W["wfx"][:], xb, start=False, stop=True
            )
        nc.scalar.activation(f[:, s0:e0], pf[:], SIG, bias=W["bf"][:])
    prod = gg.tile([128, cols], F16, tag="prod")
    nc.vector.tensor_mul(prod[:], f[:], c_ch)
    prod3 = prod[:].rearrange("p (m f) -> p m f", f=8)
    _tree_reduce8(nc, gg, prod3, m, fc_dst, "fc")


def _level_top_small(nc, psum, gp, W, n, x_l, hs, fc, c_dst, h_dst):
    """iou gates + cell update, latency-optimized for small n (<=512).

    Biases are added in PSUM via K=1 matmuls (ones-rhs) so sigmoid(i) and
    sigmoid(o) can run as one ACT op over [128, 2n].
    """
    p = psum.tile([128, 3 * n], F32, tag="pg")
    ones = W["ones"]
    for gate in range(3):
        sl = slice(gate * n, (gate + 1) * n)
        w = W["wioux"][:, gate * 128 : (gate + 1) * 128]
        nc.tensor.matmul(p[:, sl], w, x_l, start=True, stop=False)
        wh = W["wiouh"][:, gate * 128 : (gate + 1) * 128]
        nc.tensor.matmul(p[:, sl], wh, hs, start=False, stop=False)
        br = W["biourow"][:, gate * 128 : (gate + 1) * 128]
        nc.tensor.matmul(p[:, sl], br, ones[:, 0:n], start=False, stop=True)

    sio = gp.tile([128, 2 * n], F16, tag="sio")
    nc.scalar.activation(sio[:], p[:, 0 : 2 * n], SIG)
    tu = gp.tile([128, n], F16, tag="tu_s")
    nc.scalar.activation(tu[:], p[:, 2 * n : 3 * n], TANH)
    ct = gp.tile([128, n], F16, tag="ct_s")
    nc.vector.tensor_mul(ct[:], sio[:, 0:n], tu[:])
    nc.vector.tensor_add(c_dst, ct[:], fc)
    tct = gp.tile([128, n], F16, tag="tct_s")
    nc.scalar.activation(tct[:], c_dst, TANH)
    nc.vector.tensor_mul(h_dst, sio[:, n : 2 * n], tct[:])


def _build_subtree_kernel():
    nc = bacc.Bacc("TRN2", target_bir_lowering=False, debug=False, num_devices=8)

    xs = nc.dram_tensor("xs", [128, SUB_N], F16, kind="ExternalInput").ap()
    wioux_d = nc.dram_tensor("wioux", [128, 384], F16, kind="ExternalInput").ap()
    wiouh_d = nc.dram_tensor("wiouh", [128, 384], F16, kind="ExternalInput").ap()
    wfx_d = nc.dram_tensor("wfx", [128, 128], F16, kind="ExternalInput").ap()
    wfh_d = nc.dram_tensor("wfh", [128, 128], F16, kind="ExternalInput").ap()
    biou_d = nc.dram_tensor("biou", [128, 3], F32, kind="ExternalInput").ap()
    biourow_d = nc.dram_tensor("biourow", [1, 384], F16, kind="ExternalInput").ap()
    bf_d = nc.dram_tensor("bf", [128, 1], F32, kind="ExternalInput").ap()
    # out: cols [0:512) = hs4, [512:1024) = fc4  (L4 partials: h child-sum
    # and sum(f*c); the L4 iou/cell update runs on the host in fp32)
    out_d = nc.dram_tensor("out", [128, 1024], F16, kind="ExternalOutput").ap()

    with tile.TileContext(nc) as tc:
        with (
            tc.tile_pool(name="const", bufs=1) as cp,
            tc.tile_pool(name="xlo", bufs=1) as xlo_p,
            tc.tile_pool(name="x6", bufs=2) as x6_p,
            tc.tile_pool(name="leafst", bufs=2) as lf_p,
            tc.tile_pool(name="state", bufs=1) as st,
            tc.tile_pool(name="gates", bufs=2) as gp,
            tc.tile_pool(name="gg", bufs=1) as gg,
            tc.tile_pool(name="gf", bufs=2) as gf,
            tc.tile_pool(name="psum", bufs=2, space="PSUM") as psum,
        ):
            # --- constants: the tiny weights for the first leaf matmuls go on
            # the fast HWDGE queue FIRST (so they land before the x stream),
            # everything else on the gpsimd queue.
            W = {}

            def load_const(name, dram, shape, dt, engine):
                t = cp.tile(shape, dt, tag=name)
                engine.dma_start(t[:], dram)
                W[name] = t

            load_const("wioux", wioux_d, [128, 384], F16, nc.sync)
            load_const("biou", biou_d, [128, 3], F32, nc.sync)

            # First leaf chunk half as a 512 + 1536 col split, on the fast
            # HWDGE (sync) queue right behind the first weights, so the very
            # first gate matmuls/ACTs wait on only 128KB of x.
            x6_tiles = {}
            x6_first = [
                x6_p.tile([128, 512], F16, tag="x6a", name="x6f_0"),
                x6_p.tile([128, 512], F16, tag="x6c", name="x6f_1"),
                x6_p.tile([128, 1024], F16, tag="x6b", name="x6f_2"),
            ]
            nc.sync.dma_start(x6_first[0][:], xs[:, LEAF_OFF : LEAF_OFF + 512])
            # the scalar queue is also a HWDGE and idle at start — land the
            # next pieces in parallel with the first
            nc.scalar.dma_start(
                x6_first[1][:], xs[:, LEAF_OFF + 512 : LEAF_OFF + 1024]
            )
            nc.sync.dma_start(
                x6_first[2][:], xs[:, LEAF_OFF + 1024 : LEAF_OFF + 2048]
            )

            load_const("wfx", wfx_d, [128, 128], F16, nc.gpsimd)
            load_const("wfh", wfh_d, [128, 128], F16, nc.gpsimd)
            load_const("bf", bf_d, [128, 1], F32, nc.gpsimd)
            ones = cp.tile([1, 512], F16, tag="ones")
            nc.vector.memset(ones[:], 1.0)
            W["ones"] = ones
            # Dummy ACT so the sigmoid/tanh table load (~1.3us) happens while
            # the first x DMA is still in flight, off the critical path.
            warm = cp.tile([1, 2], F16, tag="warm")
            nc.scalar.activation(warm[:], ones[:, 0:2], SIG)

            load_const("wiouh", wiouh_d, [128, 384], F16, nc.gpsimd)
            load_const("biourow", biourow_d, [1, 384], F16, nc.gpsimd)

            x6_tiles[(0, 1)] = x6_p.tile([128, 2048], F16, tag="x6", name="x6_0_1")
            nc.sync.dma_start(
                x6_tiles[(0, 1)][:], xs[:, LEAF_OFF + 2048 : LEAF_OFF + 4096]
            )

            # x for levels L1..L5 (cols 0..4681), persistent; split into
            # pieces on the gpsimd queue so early L5-gate readers unblock
            # before the whole 1.2MB has landed.
            x15 = xlo_p.tile([128, LEAF_OFF], F16)
            for s, e in ((L5_OFF, L5_OFF + 2048), (L5_OFF + 2048, LEAF_OFF),
                         (0, L5_OFF)):
                nc.gpsimd.dma_start(x15[:, s:e], xs[:, s:e])

            # persistent state/partials
            hs5 = st.tile([128, 4096], F16, tag="hs5")
            fc5 = st.tile([128, 4096], F16, tag="fc5")
            c5 = st.tile([128, 4096], F16, tag="c5")
            h5 = st.tile([128, 4096], F16, tag="h5")
            hs4 = st.tile([128, 512], F16, tag="hs4")
            fc4 = st.tile([128, 512], F16, tag="fc4")

            def _iou_psum(st5, n, gate):
                sl = slice(st5, st5 + n)
                x_l = x15[:, L5_OFF + sl.start : L5_OFF + sl.stop]
                p = psum.tile([128, n], F32, tag="pg", name=f"p5_{st5}_{gate}")
                w = W["wioux"][:, gate * 128 : (gate + 1) * 128]
                for s in range(0, n, 512):
                    e = min(n, s + 512)
                    nc.tensor.matmul(
                        p[:, s:e], w, x_l[:, s:e], start=True, stop=False
                    )
                wh = W["wiouh"][:, gate * 128 : (gate + 1) * 128]
                for s in range(0, n, 512):
                    e = min(n, s + 512)
                    nc.tensor.matmul(
                        p[:, s:e], wh, hs5[:, sl][:, s:e], start=False, stop=True
                    )
                return p

            def l5top_p1s(st5, n):
                # sigmoid(i), tanh(u), c = si*tu + fc for L5 nodes [st5, st5+n)
                sl = slice(st5, st5 + n)
                pi = _iou_psum(st5, n, 0)
                pu = _iou_psum(st5, n, 2)
                si = gp.tile([128, n], F16, tag="si5", name=f"si5_{st5}")
                nc.scalar.activation(si[:], pi[:], SIG, bias=W["biou"][:, 0:1])
                tu = gp.tile([128, n], F16, tag="tu5", name=f"tu5_{st5}")
                nc.scalar.activation(tu[:], pu[:], TANH, bias=W["biou"][:, 2:3])
                ct = gp.tile([128, n], F16, tag="ct5", name=f"ct5_{st5}")
                nc.vector.tensor_mul(ct[:], si[:], tu[:])
                nc.vector.tensor_add(c5[:, sl], ct[:], fc5[:, sl])

            def l5top_p2s(st5, n):
                # sigmoid(o), tanh(c), h = so*tc for L5 nodes [st5, st5+n)
                sl = slice(st5, st5 + n)
                po = _iou_psum(st5, n, 1)
                so = gp.tile([128, n], F16, tag="so5", name=f"so5_{st5}")
                nc.scalar.activation(so[:], po[:], SIG, bias=W["biou"][:, 1:2])
                tct = gp.tile([128, n], F16, tag="tct5", name=f"tct5_{st5}")
                nc.scalar.activation(tct[:], c5[:, sl], TANH)
                nc.vector.tensor_mul(h5[:, sl], so[:], tct[:])

            def l4gate_r(pstart, m):
                sl = slice(pstart * 8, (pstart + m) * 8)
                _gate_group(
                    nc, psum, gg, gf, W, m, c5[:, sl], h5[:, sl],
                    x15[:, L4_OFF + pstart : L4_OFF + pstart + m],
                    hs4[:, pstart : pstart + m], fc4[:, pstart : pstart + m],
                )

            def ship4(start, n):
                # stream this piece's L4 partials out while compute continues
                sl = slice(start, start + n)
                nc.sync.dma_start(out_d[:, sl.start : sl.stop], hs4[:, sl])
                nc.sync.dma_start(
                    out_d[:, 512 + sl.start : 512 + sl.stop], fc4[:, sl]
                )

            # ---- Phase 1: leaves fused with L5 forget-gates/child-sums.
            # The L5 gate stage for chunk ch-1 is issued after the leaves of
            # chunk ch (one full chunk of slack), and the L5-top / L4 stages
            # are interleaved with >=1 chunk of slack, so every inserted
            # chain's inputs are long ready and the ACT stream never stalls.
            leaf_states = {}
            leaf_deferred = {}

            def leaf_chunk(ch):
                # Chunk 0's first half is processed as two 1024-col pieces so
                # the very first matmuls and ACTs only wait on 256KB of x.
                lc = lf_p.tile([128, 4096], F16, tag="lc", name=f"lc{ch}")
                lh = lf_p.tile([128, 4096], F16, tag="lh", name=f"lh{ch}")
                leaf_states[ch] = (lc, lh)
                halves = []
                if ch == 0:
                    halves = [
                        (0, [(x6_first[0], 0, 512), (x6_first[1], 512, 512),
                             (x6_first[2], 1024, 1024)]),
                        (2048, [(x6_tiles[(0, 1)], 2048, 2048)]),
                    ]
                else:
                    for g in range(2):
                        t = x6_p.tile(
                            [128, 2048], F16, tag="x6", name=f"x6_{ch}_{g}"
                        )
                        base = LEAF_OFF + ch * 4096 + g * 2048
                        nc.sync.dma_start(t[:], xs[:, base : base + 2048])
                        halves.append((g * 2048, [(t, g * 2048, 2048)]))
                deferred = []
                for hstart, pieces in halves:
                    hsl = slice(hstart, hstart + 2048)
                    si = gp.tile([128, 2048], F16, tag="si", name=f"si{ch}_{hstart}")
                    tu = gp.tile([128, 2048], F16, tag="tu", name=f"tu{ch}_{hstart}")
                    so = gp.tile([128, 2048], F16, tag="so", name=f"so{ch}_{hstart}")
                    for x6t, st_, w in pieces:
                        psl = slice(st_ - hstart, st_ - hstart + w)

                        def gate_psum(gate, name):
                            p = psum.tile([128, w], F32, tag="pg", name=name)
                            wm = W["wioux"][:, gate * 128 : (gate + 1) * 128]
                            for s in range(0, w, 512):
                                nc.tensor.matmul(
                                    p[:, s : s + 512], wm, x6t[:, s : s + 512],
                                    start=True, stop=True,
                                )
                            return p

                        pi = gate_psum(0, f"pi6_{ch}_{st_}")
                        pu = gate_psum(2, f"pu6_{ch}_{st_}")
                        nc.scalar.activation(
                            si[:, psl], pi[:], SIG, bias=W["biou"][:, 0:1]
                        )
                        nc.scalar.activation(
                            tu[:, psl], pu[:], TANH, bias=W["biou"][:, 2:3]
                        )
                        po = gate_psum(1, f"po6_{ch}_{st_}")
                        nc.scalar.activation(
                            so[:, psl], po[:], SIG, bias=W["biou"][:, 1:2]
                        )
                    nc.vector.tensor_mul(lc[:, hsl], si[:], tu[:])
                    tct = gp.tile(
                        [128, 2048], F16, tag="tct", name=f"tct{ch}_{hstart}"
                    )
                    if hstart == 0 or ch in QUINTIC_BOTH:
                        # half 0: all-DVE tanh(c) via the quintic (|c|<1:
                        # tanh(x) ~ x(1 + q(A5 + B5 q)), q=x^2), so lh[0:2048]
                        # never waits on ACT and the next gate-group's
                        # matmuls can start early.
                        lcs = lc[:, hsl]
                        q = gp.tile([128, 2048], F16, tag="q", name=f"q{ch}")
                        qt = gp.tile([128, 2048], F16, tag="qt", name=f"qt{ch}")
                        nc.vector.tensor_mul(q[:], lcs, lcs)
                        nc.vector.tensor_scalar(qt[:], q[:], B5, A5, MUL, ADD)
                        nc.vector.tensor_mul(tct[:], qt[:], q[:])
                        nc.vector.tensor_scalar(qt[:], tct[:], 1.0, None, ADD)
                        nc.vector.tensor_mul(tct[:], lcs, qt[:])
                        nc.vector.tensor_mul(lh[:, hsl], so[:], tct[:])
                    else:
                        # half 1: tanh(c) on ACT, deferred until after the
                        # previous chunk's forget-gate ACT ops so the DVE's
                        # lc mul has a full pipeline stage of slack.
                        deferred.append((so, tct, hsl))
                leaf_deferred[ch] = deferred

            def leaf_finish(ch):
                lc, lh = leaf_states[ch]
                for so, tct, hsl in leaf_deferred.pop(ch):
                    nc.scalar.activation(tct[:], lc[:, hsl], TANH)
                    nc.vector.tensor_mul(lh[:, hsl], so[:], tct[:])

            def l5_gates(ch):
                lc, lh = leaf_states.pop(ch)
                _gate_group(
                    nc, psum, gg, gf, W, 512, lc[:], lh[:],
                    x15[:, L5_OFF + ch * 512 : L5_OFF + (ch + 1) * 512],
                    hs5[:, ch * 512 : (ch + 1) * 512],
                    fc5[:, ch * 512 : (ch + 1) * 512],
                )

            def l5_gates_part(ch, half):
                # half-chunk variant (256 parents) for the endgame pipeline
                lc, lh = leaf_states[ch]
                coff = half * 2048
                pstart = ch * 512 + half * 256
                _gate_group(
                    nc, psum, gg, gf, W, 256,
                    lc[:, coff : coff + 2048], lh[:, coff : coff + 2048],
                    x15[:, L5_OFF + pstart : L5_OFF + pstart + 256],
                    hs5[:, pstart : pstart + 256],
                    fc5[:, pstart : pstart + 256],
                )
                if half == 1:
                    leaf_states.pop(ch)

            def l5_gates_part(ch, half):
                # half-chunk variant (256 parents) for the endgame pipeline
                lc, lh = leaf_states[ch]
                coff = half * 2048
                pstart = ch * 512 + half * 256
                _gate_group(
                    nc, psum, gg, gf, W, 256,
                    lc[:, coff : coff + 2048], lh[:, coff : coff + 2048],
                    x15[:, L5_OFF + pstart : L5_OFF + pstart + 256],
                    hs5[:, pstart : pstart + 256],
                    fc5[:, pstart : pstart + 256],
                )
                if half == 1:
                    leaf_states.pop(ch)

            # ---- Main loop. Engine queues are strict FIFO, so every
            # inserted stage (L5 tops, L4 gates/tops) is scheduled >=1 chunk
            # after its producers — inserted ops never head-of-line-block
            # the leaf ACT stream.
            for ch in range(N_CHUNKS):
                leaf_chunk(ch)
                if ch == 5:
                    l5top_p2s(0, 1024)
                elif ch == 6:
                    l5top_p2s(1024, 1024)
                elif ch == 7:
                    l5top_p2s(2048, 1024)
                if ch >= 1:
                    l5_gates(ch - 1)
                leaf_finish(ch)
                if ch == 4:
                    l5top_p1s(0, 1024)
                elif ch == 5:
                    l5top_p1s(1024, 1024)
                elif ch == 6:
                    l5top_p1s(2048, 1024)
                    l4gate_r(0, 128)
                elif ch == 7:
                    l4gate_r(128, 128)
                    ship4(0, 256)

            # ---- tail: only the chunk-7-dependent chain remains; ready
            # work is interleaved between the dependent stages so each has
            # slack when the ACT queue (strict FIFO) reaches it.
            l5_gates_part(7, 0)        # parents 3584:3840
            l5top_p1s(3072, 512)       # gates(6) finished a stage ago
            l4gate_r(256, 128)         # children 2048:3072 (ready)
            l5_gates_part(7, 1)        # parents 3840:4096
            l5top_p2s(3072, 512)
            l5top_p1s(3584, 256)
            l4gate_r(384, 64)          # children 3072:3584
            l5top_p2s(3584, 256)
            l5top_p1s(3840, 256)
            l4gate_r(448, 32)          # children 3584:3840
            ship4(256, 128)
            l5top_p2s(3840, 256)
            l4gate_r(480, 32)          # children 3840:4096
            ship4(384, 128)

    nc.compile()
    return nc


_NC_CACHE = None


def _get_nc():
    global _NC_CACHE
    if _NC_CACHE is None:
        _NC_CACHE = _build_subtree_kernel()
    return _NC_CACHE


def _sigmoid(x):
    return 1.0 / (1.0 + np.exp(-x))


def kernel(
    x, W_ioux, b_ioux, W_iouh, b_iouh, W_fx, b_fx, W_fh, b_fh, branch, depth
):
    global LAST_RESULTS
    assert int(branch) == BRANCH and int(depth) == DEPTH

    x = np.asarray(x, np.float32)
    W_ioux = np.asarray(W_ioux, np.float32)
    b_ioux = np.asarray(b_ioux, np.float32)
    W_iouh = np.asarray(W_iouh, np.float32)
    b_iouh = np.asarray(b_iouh, np.float32)
    W_fx = np.asarray(W_fx, np.float32)
    b_fx = np.asarray(b_fx, np.float32)
    W_fh = np.asarray(W_fh, np.float32)
    b_fh = np.asarray(b_fh, np.float32)

    wioux = np.ascontiguousarray(W_ioux.T.astype(np.float16))
    wiouh = np.ascontiguousarray(W_iouh.T.astype(np.float16))
    wfx = np.ascontiguousarray(W_fx.T.astype(np.float16))
    wfh = np.ascontiguousarray(W_fh.T.astype(np.float16))
    biou_full = b_ioux + b_iouh
    biou = np.ascontiguousarray(biou_full.reshape(3, 128).T.astype(np.float32))
    biourow = np.ascontiguousarray(biou_full.reshape(1, 384).astype(np.float16))
    bf = np.ascontiguousarray((b_fx + b_fh).reshape(128, 1).astype(np.float32))

    off = lambda l: (BRANCH**l - 1) // (BRANCH - 1)
    in_maps = []
    for c in range(BRANCH):
        parts = []
        for l in range(1, DEPTH + 1):
            sz = BRANCH ** (l - 1)
            parts.append(x[off(l) + c * sz : off(l) + (c + 1) * sz])
        xs_c = np.ascontiguousarray(
            np.concatenate(parts, axis=0).T.astype(np.float16)
        )
        in_maps.append(
            {
                "xs": xs_c,
                "wioux": wioux,
                "wiouh": wiouh,
                "wfx": wfx,
                "wfh": wfh,
                "biou": biou,
                "biourow": biourow,
                "bf": bf,
            }
        )

    nc = _get_nc()
    trace = os.environ.get("TREELSTM_TRACE") == "1"
    res = run_bass_kernel_spmd(nc, in_maps, core_ids=list(range(8)), trace=trace)
    LAST_RESULTS = res

    # L4 partials from the device: [8, 512, 128] (node-major)
    hs4 = np.stack(
        [res.results[s]["out"][:, 0:512].T.astype(np.float32) for s in range(8)]
    )
    fc4 = np.stack(
        [res.results[s]["out"][:, 512:1024].T.astype(np.float32) for s in range(8)]
    )

    # L4 iou + cell update in fp32 on the host (the forget gates and child
    # sums for L4 ran on the device)
    xs_4 = np.stack(
        [x[off(4) + s * 512 : off(4) + (s + 1) * 512] for s in range(8)]
    )  # [8, 512, 128]
    iou = xs_4 @ W_ioux.T + b_ioux + hs4 @ W_iouh.T + b_iouh
    i, o, u = iou[..., 0:128], iou[..., 128:256], iou[..., 256:384]
    c_prev = _sigmoid(i) * np.tanh(u) + fc4
    h_prev = _sigmoid(o) * np.tanh(c_prev)

    # L3/L2/L1 of each subtree on the host (levels of 64/8/1 nodes), matching
    # the reference recursion exactly in fp32.
    for l in (3, 2, 1):
        n = BRANCH ** (l - 1)
        c_ch = c_prev.reshape(8, n, BRANCH, MEM)
        h_ch = h_prev.reshape(8, n, BRANCH, MEM)
        xs_l = np.stack(
            [x[off(l) + s * n : off(l) + (s + 1) * n] for s in range(8)]
        )  # [8, n, 128]
        h_sum = h_ch.sum(axis=2)
        f = _sigmoid(
            np.einsum("snbm,km->snbk", h_ch, W_fh)
            + b_fh
            + (xs_l @ W_fx.T + b_fx)[:, :, None, :]
        )
        fc_sum = (f * c_ch).sum(axis=2)
        iou = xs_l @ W_ioux.T + b_ioux + h_sum @ W_iouh.T + b_iouh
        i, o, u = iou[..., 0:128], iou[..., 128:256], iou[..., 256:384]
        c_prev = _sigmoid(i) * np.tanh(u) + fc_sum
        h_prev = _sigmoid(o) * np.tanh(c_prev)

    c_ch = c_prev[:, 0, :]  # [8, 128] — the root's children
    h_ch = h_prev[:, 0, :]

    # Root node on host (fp32), matching reference node_forward.
    x0 = x[0:1]  # [1,128]
    h_sum = h_ch.sum(axis=0, keepdims=True)  # [1,128]
    f = _sigmoid(h_ch @ W_fh.T + b_fh + (x0 @ W_fx.T + b_fx))  # [8,128]
    fc_sum = (f * c_ch).sum(axis=0, keepdims=True)  # [1,128]
    iou = x0 @ W_ioux.T + b_ioux + h_sum @ W_iouh.T + b_iouh  # [1,384]
    i, o, u = iou[:, 0:128], iou[:, 128:256], iou[:, 256:384]
    c_root = _sigmoid(i) * np.tanh(u) + fc_sum
    h_root = _sigmoid(o) * np.tanh(c_root)
    return (c_root.astype(np.float32), h_root.astype(np.float32))


# revision 41
# speedup vs baseline: 1.0324x; 1.0324x over previous
"""Child-Sum TreeLSTM over a complete 8-ary tree (depth 6, 299593 nodes) on
8 Trainium2 NeuronCores.

Sharding: the 8 independent subtrees under the root go one-per-core; each core
runs the bottom-up sweep for levels L6 (leaves) .. L4 of its subtree and ships
the L4 partials (hs4 = sum of child h, fc4 = sum of f*c: 512 nodes) back.
The L4 iou/cell update plus the tiny latency-bound tip of the tree (L3/L2/L1
per subtree + the root) run vectorized in fp32 on the host — everything
level-5-down (99.8% of nodes, all 299,592 child-edge forget gates and child
sums) stays on the device.  This removes a ~25us serial small-op tail from
the device critical path and improves accuracy (fp32 tops).

Device layout: feature-major ([128 features on partitions, nodes on free
dim]), fp16 everywhere, matmuls accumulate in fp32 PSUM.  The kernel is
ScalarE(ACT)-bound: every sigmoid/tanh costs 1 column/cycle at 1.2 GHz and
the model needs 4 activations per node + 1 per child edge.  Two mitigations:
  - the leaf tanh(c) (the only activation whose argument is provably in
    [-1, 1]: c_leaf = sigmoid(i)*tanh(u)) is split: the first SPLIT_TCT
    columns per chunk run on ACT, the rest as a quintic odd polynomial on the
    (less loaded) vector engine;
  - elementwise DVE ops are batched to 4096-wide ops to amortize per-op
    overhead, and the per-child forget path accumulates W_fh@h_child and
    W_fx@x_parent into one PSUM via a broadcast-rhs matmul (PE has slack).
"""

import os

import numpy as np

import concourse.tile as tile
from concourse import bacc, mybir
from concourse.bass_utils import run_bass_kernel_spmd

F16 = mybir.dt.float16
F32 = mybir.dt.float32
SIG = mybir.ActivationFunctionType.Sigmoid
TANH = mybir.ActivationFunctionType.Tanh
MUL = mybir.AluOpType.mult
ADD = mybir.AluOpType.add

BRANCH = 8
DEPTH = 6
MEM = 128
IN_DIM = 128
N_NODES = (BRANCH ** (DEPTH + 1) - 1) // (BRANCH - 1)  # 299593

# Per-subtree (local) levels L1..L6: sizes 8^(L-1), offsets into the per-core
# x tensor (levels concatenated in order L1..L6).
LVL_SIZES = [BRANCH**i for i in range(DEPTH)]  # [1, 8, 64, 512, 4096, 32768]
LVL_OFF = [sum(LVL_SIZES[:i]) for i in range(DEPTH)]  # [0,1,9,73,585,4681]
SUB_N = sum(LVL_SIZES)  # 37449

LEAF_OFF = LVL_OFF[5]  # 4681
L5_OFF = LVL_OFF[4]  # 585
L4_OFF = LVL_OFF[3]  # 73
N_CHUNKS = 8  # leaf/L5 fused chunks: 512 L5-parents (4096 leaves) each
# chunks whose SECOND half also uses the DVE quintic (ACT<->DVE balance knob)
QUINTIC_BOTH = (4,)

# Leaf tanh(c) split: columns [0:SPLIT_TCT) per 4096-chunk on ACT, the rest
# via the DVE quintic tanh(x) ~ x*(1 + A5 x^2 + B5 x^4) (max err 2.1e-3 on
# [-1,1]; |c_leaf| < 1 always since c_leaf = sigmoid*tanh).
SPLIT_TCT = 2048
A5 = -0.32159217
B5 = 0.08530049

LAST_RESULTS = None  # stash for test harness introspection


def _tree_reduce8(nc, pool, src3, m, dst, tag):
    """dst[128, m] = sum over last axis of src3 [128, m, 8] (fp16 tree adds)."""
    t1 = pool.tile([128, m * 4], F16, tag=tag + "_t1")
    t1v = t1[:].rearrange("p (m f) -> p m f", f=4)
    nc.vector.tensor_add(t1v, src3[:, :, 0:4], src3[:, :, 4:8])
    t2 = pool.tile([128, m * 2], F16, tag=tag + "_t2")
    t2v = t2[:].rearrange("p (m f) -> p m f", f=2)
    nc.vector.tensor_add(t2v, t1v[:, :, 0:2], t1v[:, :, 2:4])
    dstv = dst.rearrange("p (m f) -> p m f", f=1)
    nc.vector.tensor_add(dstv, t2v[:, :, 0:1], t2v[:, :, 1:2])


def _gate_group(nc, psum, gg, gf, W, m, c_ch, h_ch, x_par, hs_dst, fc_dst):
    """Forget gates + child sums for m parents (8m children).

    c_ch/h_ch: [128, 8m] fp16 child states; x_par: [128, m] fp16 parent x.
    Writes hs_dst/fc_dst [128, m] (h child-sum, sum of f*c).
    DVE ops run at full 8m width; PE/ACT work in <=2048-col sub-pieces
    (PSUM bank limit).
    """
    cols = 8 * m
    # h child-sum first: it doesn't depend on the forget sigmoid, so the
    # DVE computes it while ACT runs f, and it feeds the next level's
    # (critical) iou_h matmuls earlier.
    hch3 = h_ch.rearrange("p (m f) -> p m f", f=8)
    _tree_reduce8(nc, gg, hch3, m, hs_dst, "hs")
    f = gf.tile([128, cols], F16, tag="f")
    for s0 in range(0, cols, 2048):
        e0 = min(cols, s0 + 2048)
        pf = psum.tile([128, e0 - s0], F32, tag="pg", name=f"pf_{s0}")
        # weight-major order: all wfh matmuls, then all wfx, so LDWEIGHTS is
        # not re-issued per matmul.
        for s in range(s0, e0, 512):
            e = min(e0, s + 512)
            nc.tensor.matmul(
                pf[:, s - s0 : e - s0], W["wfh"][:], h_ch[:, s:e],
                start=True, stop=False,
            )
        for s in range(s0, e0, 512):
            e = min(e0, s + 512)
            xb = (
                x_par[:, s // 8 : e // 8]
                .rearrange("p (m o) -> p m o", o=1)
                .broadcast_to([128, (e - s) // 8, 8])
            )
            nc.tensor.matmul(
                pf[:, s - s0 : e - s0], W["wfx"][:], xb, start=False, stop=True
            )
        nc.scalar.activation(f[:, s0:e0], pf[:], SIG, bias=W["bf"][:])
    prod = gg.tile([128, cols], F16, tag="prod")
    nc.vector.tensor_mul(prod[:], f[:], c_ch)
    prod3 = prod[:].rearrange("p (m f) -> p m f", f=8)
    _tree_reduce8(nc, gg, prod3, m, fc_dst, "fc")


def _level_top_small(nc, psum, gp, W, n, x_l, hs, fc, c_dst, h_dst):
    """iou gates + cell update, latency-optimized for small n (<=512).

    Biases are added in PSUM via K=1 matmuls (ones-rhs) so sigmoid(i) and
    sigmoid(o) can run as one ACT op over [128, 2n].
    """
    p = psum.tile([128, 3 * n], F32, tag="pg")
    ones = W["ones"]
    for gate in range(3):
        sl = slice(gate * n, (gate + 1) * n)
        w = W["wioux"][:, gate * 128 : (gate + 1) * 128]
        nc.tensor.matmul(p[:, sl], w, x_l, start=True, stop=False)
        wh = W["wiouh"][:, gate * 128 : (gate + 1) * 128]
        nc.tensor.matmul(p[:, sl], wh, hs, start=False, stop=False)
        br = W["biourow"][:, gate * 128 : (gate + 1) * 128]
        nc.tensor.matmul(p[:, sl], br, ones[:, 0:n], start=False, stop=True)

    sio = gp.tile([128, 2 * n], F16, tag="sio")
    nc.scalar.activation(sio[:], p[:, 0 : 2 * n], SIG)
    tu = gp.tile([128, n], F16, tag="tu_s")
    nc.scalar.activation(tu[:], p[:, 2 * n : 3 * n], TANH)
    ct = gp.tile([128, n], F16, tag="ct_s")
    nc.vector.tensor_mul(ct[:], sio[:, 0:n], tu[:])
    nc.vector.tensor_add(c_dst, ct[:], fc)
    tct = gp.tile([128, n], F16, tag="tct_s")
    nc.scalar.activation(tct[:], c_dst, TANH)
    nc.vector.tensor_mul(h_dst, sio[:, n : 2 * n], tct[:])


def _build_subtree_kernel():
    nc = bacc.Bacc("TRN2", target_bir_lowering=False, debug=False, num_devices=8)

    xs = nc.dram_tensor("xs", [128, SUB_N], F16, kind="ExternalInput").ap()
    wioux_d = nc.dram_tensor("wioux", [128, 384], F16, kind="ExternalInput").ap()
    wiouh_d = nc.dram_tensor("wiouh", [128, 384], F16, kind="ExternalInput").ap()
    wfx_d = nc.dram_tensor("wfx", [128, 128], F16, kind="ExternalInput").ap()
    wfh_d = nc.dram_tensor("wfh", [128, 128], F16, kind="ExternalInput").ap()
    biou_d = nc.dram_tensor("biou", [128, 3], F32, kind="ExternalInput").ap()
    biourow_d = nc.dram_tensor("biourow", [1, 384], F16, kind="ExternalInput").ap()
    bf_d = nc.dram_tensor("bf", [128, 1], F32, kind="ExternalInput").ap()
    # out: cols [0:512) = hs4, [512:1024) = fc4  (L4 partials: h child-sum
    # and sum(f*c); the L4 iou/cell update runs on the host in fp32)
    out_d = nc.dram_tensor("out", [128, 1024], F16, kind="ExternalOutput").ap()

    with tile.TileContext(nc) as tc:
        with (
            tc.tile_pool(name="const", bufs=1) as cp,
            tc.tile_pool(name="xlo", bufs=1) as xlo_p,
            tc.tile_pool(name="x6", bufs=2) as x6_p,
            tc.tile_pool(name="leafst", bufs=2) as lf_p,
            tc.tile_pool(name="state", bufs=1) as st,
            tc.tile_pool(name="gates", bufs=2) as gp,
            tc.tile_pool(name="gg", bufs=1) as gg,
            tc.tile_pool(name="gf", bufs=2) as gf,
            tc.tile_pool(name="psum", bufs=2, space="PSUM") as psum,
        ):
            # --- constants: the tiny weights for the first leaf matmuls go on
            # the fast HWDGE queue FIRST (so they land before the x stream),
            # everything else on the gpsimd queue.
            W = {}

            def load_const(name, dram, shape, dt, engine):
                t = cp.tile(shape, dt, tag=name)
                engine.dma_start(t[:], dram)
                W[name] = t

            load_const("wioux", wioux_d, [128, 384], F16, nc.sync)
            load_const("biou", biou_d, [128, 3], F32, nc.sync)

            # First leaf chunk half as a 512 + 1536 col split, on the fast
            # HWDGE (sync) queue right behind the first weights, so the very
            # first gate matmuls/ACTs wait on only 128KB of x.
            x6_tiles = {}
            x6_first = [
                x6_p.tile([128, 512], F16, tag="x6a", name="x6f_0"),
                x6_p.tile([128, 512], F16, tag="x6c", name="x6f_1"),
                x6_p.tile([128, 1024], F16, tag="x6b", name="x6f_2"),
            ]
            nc.sync.dma_start(x6_first[0][:], xs[:, LEAF_OFF : LEAF_OFF + 512])
            # the scalar queue is also a HWDGE and idle at start — land the
            # next pieces in parallel with the first
            nc.scalar.dma_start(
                x6_first[1][:], xs[:, LEAF_OFF + 512 : LEAF_OFF + 1024]
            )
            nc.sync.dma_start(
                x6_first[2][:], xs[:, LEAF_OFF + 1024 : LEAF_OFF + 2048]
            )

            load_const("wfx", wfx_d, [128, 128], F16, nc.gpsimd)
            load_const("wfh", wfh_d, [128, 128], F16, nc.gpsimd)
            load_const("bf", bf_d, [128, 1], F32, nc.gpsimd)
            ones = cp.tile([1, 512], F16, tag="ones")
            nc.vector.memset(ones[:], 1.0)
            W["ones"] = ones
            # Dummy ACT so the sigmoid/tanh table load (~1.3us) happens while
            # the first x DMA is still in flight, off the critical path.
            warm = cp.tile([1, 2], F16, tag="warm")
            nc.scalar.activation(warm[:], ones[:, 0:2], SIG)

            load_const("wiouh", wiouh_d, [128, 384], F16, nc.gpsimd)
            load_const("biourow", biourow_d, [1, 384], F16, nc.gpsimd)

            x6_tiles[(0, 1)] = x6_p.tile([128, 2048], F16, tag="x6", name="x6_0_1")
            nc.sync.dma_start(
                x6_tiles[(0, 1)][:], xs[:, LEAF_OFF + 2048 : LEAF_OFF + 4096]
            )

            # x for levels L1..L5 (cols 0..4681), persistent; split into
            # pieces on the gpsimd queue so early L5-gate readers unblock
            # before the whole 1.2MB has landed.
            x15 = xlo_p.tile([128, LEAF_OFF], F16)
            for s, e in ((L5_OFF, L5_OFF + 2048), (L5_OFF + 2048, LEAF_OFF),
                         (0, L5_OFF)):
                nc.gpsimd.dma_start(x15[:, s:e], xs[:, s:e])

            # persistent state/partials
            hs5 = st.tile([128, 4096], F16, tag="hs5")
            fc5 = st.tile([128, 4096], F16, tag="fc5")
            c5 = st.tile([128, 4096], F16, tag="c5")
            h5 = st.tile([128, 4096], F16, tag="h5")
            hs4 = st.tile([128, 512], F16, tag="hs4")
            fc4 = st.tile([128, 512], F16, tag="fc4")

            def _iou_psum(st5, n, gate):
                sl = slice(st5, st5 + n)
                x_l = x15[:, L5_OFF + sl.start : L5_OFF + sl.stop]
                p = psum.tile([128, n], F32, tag="pg", name=f"p5_{st5}_{gate}")
                w = W["wioux"][:, gate * 128 : (gate + 1) * 128]
                for s in range(0, n, 512):
                    e = min(n, s + 512)
                    nc.tensor.matmul(
                        p[:, s:e], w, x_l[:, s:e], start=True, stop=False
                    )
                wh = W["wiouh"][:, gate * 128 : (gate + 1) * 128]
                for s in range(0, n, 512):
                    e = min(n, s + 512)
                    nc.tensor.matmul(
                        p[:, s:e], wh, hs5[:, sl][:, s:e], start=False, stop=True
                    )
                return p

            def l5top_p1s(st5, n):
                # sigmoid(i), tanh(u), c = si*tu + fc for L5 nodes [st5, st5+n)
                sl = slice(st5, st5 + n)
                pi = _iou_psum(st5, n, 0)
                pu = _iou_psum(st5, n, 2)
                si = gp.tile([128, n], F16, tag="si5", name=f"si5_{st5}")
                nc.scalar.activation(si[:], pi[:], SIG, bias=W["biou"][:, 0:1])
                tu = gp.tile([128, n], F16, tag="tu5", name=f"tu5_{st5}")
                nc.scalar.activation(tu[:], pu[:], TANH, bias=W["biou"][:, 2:3])
                ct = gp.tile([128, n], F16, tag="ct5", name=f"ct5_{st5}")
                nc.vector.tensor_mul(ct[:], si[:], tu[:])
                nc.vector.tensor_add(c5[:, sl], ct[:], fc5[:, sl])

            def l5top_p2s(st5, n):
                # sigmoid(o), tanh(c), h = so*tc for L5 nodes [st5, st5+n)
                sl = slice(st5, st5 + n)
                po = _iou_psum(st5, n, 1)
                so = gp.tile([128, n], F16, tag="so5", name=f"so5_{st5}")
                nc.scalar.activation(so[:], po[:], SIG, bias=W["biou"][:, 1:2])
                tct = gp.tile([128, n], F16, tag="tct5", name=f"tct5_{st5}")
                nc.scalar.activation(tct[:], c5[:, sl], TANH)
                nc.vector.tensor_mul(h5[:, sl], so[:], tct[:])

            def l4gate_r(pstart, m):
                sl = slice(pstart * 8, (pstart + m) * 8)
                _gate_group(
                    nc, psum, gg, gf, W, m, c5[:, sl], h5[:, sl],
                    x15[:, L4_OFF + pstart : L4_OFF + pstart + m],
                    hs4[:, pstart : pstart + m], fc4[:, pstart : pstart + m],
                )

            def ship4(start, n):
                # stream this piece's L4 partials out while compute continues
                sl = slice(start, start + n)
                nc.sync.dma_start(out_d[:, sl.start : sl.stop], hs4[:, sl])
                nc.sync.dma_start(
                    out_d[:, 512 + sl.start : 512 + sl.stop], fc4[:, sl]
                )

            # ---- Phase 1: leaves fused with L5 forget-gates/child-sums.
            # The L5 gate stage for chunk ch-1 is issued after the leaves of
            # chunk ch (one full chunk of slack), and the L5-top / L4 stages
            # are interleaved with >=1 chunk of slack, so every inserted
            # chain's inputs are long ready and the ACT stream never stalls.
            leaf_states = {}
            leaf_deferred = {}

            def leaf_chunk(ch):
                # Chunk 0's first half is processed as two 1024-col pieces so
                # the very first matmuls and ACTs only wait on 256KB of x.
                lc = lf_p.tile([128, 4096], F16, tag="lc", name=f"lc{ch}")
                lh = lf_p.tile([128, 4096], F16, tag="lh", name=f"lh{ch}")
                leaf_states[ch] = (lc, lh)
                halves = []
                if ch == 0:
                    halves = [
                        (0, [(x6_first[0], 0, 512), (x6_first[1], 512, 512),
                             (x6_first[2], 1024, 1024)]),
                        (2048, [(x6_tiles[(0, 1)], 2048, 2048)]),
                    ]
                else:
                    for g in range(2):
                        t = x6_p.tile(
                            [128, 2048], F16, tag="x6", name=f"x6_{ch}_{g}"
                        )
                        base = LEAF_OFF + ch * 4096 + g * 2048
                        nc.sync.dma_start(t[:], xs[:, base : base + 2048])
                        halves.append((g * 2048, [(t, g * 2048, 2048)]))
                deferred = []
                for hstart, pieces in halves:
                    hsl = slice(hstart, hstart + 2048)
                    si = gp.tile([128, 2048], F16, tag="si", name=f"si{ch}_{hstart}")
                    tu = gp.tile([128, 2048], F16, tag="tu", name=f"tu{ch}_{hstart}")
                    so = gp.tile([128, 2048], F16, tag="so", name=f"so{ch}_{hstart}")
                    for x6t, st_, w in pieces:
                        psl = slice(st_ - hstart, st_ - hstart + w)

                        def gate_psum(gate, name):
                            p = psum.tile([128, w], F32, tag="pg", name=name)
                            wm = W["wioux"][:, gate * 128 : (gate + 1) * 128]
                            for s in range(0, w, 512):
                                nc.tensor.matmul(
                                    p[:, s : s + 512], wm, x6t[:, s : s + 512],
                                    start=True, stop=True,
                                )
                            return p

                        pi = gate_psum(0, f"pi6_{ch}_{st_}")
                        pu = gate_psum(2, f"pu6_{ch}_{st_}")
                        nc.scalar.activation(
                            si[:, psl], pi[:], SIG, bias=W["biou"][:, 0:1]
                        )
                        nc.scalar.activation(
                            tu[:, psl], pu[:], TANH, bias=W["biou"][:, 2:3]
                        )
                        po = gate_psum(1, f"po6_{ch}_{st_}")
                        nc.scalar.activation(
                            so[:, psl], po[:], SIG, bias=W["biou"][:, 1:2]
                        )
                    nc.vector.tensor_mul(lc[:, hsl], si[:], tu[:])
                    tct = gp.tile(
                        [128, 2048], F16, tag="tct", name=f"tct{ch}_{hstart}"
                    )
                    if hstart == 0 or ch in QUINTIC_BOTH:
                        # half 0: all-DVE tanh(c) via the quintic (|c|<1:
                        # tanh(x) ~ x(1 + q(A5 + B5 q)), q=x^2), so lh[0:2048]
                        # never waits on ACT and the next gate-group's
                        # matmuls can start early.
                        lcs = lc[:, hsl]
                        q = gp.tile([128, 2048], F16, tag="q", name=f"q{ch}")
                        qt = gp.tile([128, 2048], F16, tag="qt", name=f"qt{ch}")
                        nc.vector.tensor_mul(q[:], lcs, lcs)
                        nc.vector.tensor_scalar(qt[:], q[:], B5, A5, MUL, ADD)
                        nc.vector.tensor_mul(tct[:], qt[:], q[:])
                        nc.vector.tensor_scalar(qt[:], tct[:], 1.0, None, ADD)
                        nc.vector.tensor_mul(tct[:], lcs, qt[:])
                        nc.vector.tensor_mul(lh[:, hsl], so[:], tct[:])
                    else:
                        # half 1: tanh(c) on ACT, deferred until after the
                        # previous chunk's forget-gate ACT ops so the DVE's
                        # lc mul has a full pipeline stage of slack.
                        deferred.append((so, tct, hsl))
                leaf_deferred[ch] = deferred

            def leaf_finish(ch):
                lc, lh = leaf_states[ch]
                for so, tct, hsl in leaf_deferred.pop(ch):
                    nc.scalar.activation(tct[:], lc[:, hsl], TANH)
                    nc.vector.tensor_mul(lh[:, hsl], so[:], tct[:])

            def l5_gates(ch):
                lc, lh = leaf_states.pop(ch)
                _gate_group(
                    nc, psum, gg, gf, W, 512, lc[:], lh[:],
                    x15[:, L5_OFF + ch * 512 : L5_OFF + (ch + 1) * 512],
                    hs5[:, ch * 512 : (ch + 1) * 512],
                    fc5[:, ch * 512 : (ch + 1) * 512],
                )

            def l5_gates_part(ch, half):
                # half-chunk variant (256 parents) for the endgame pipeline
                lc, lh = leaf_states[ch]
                coff = half * 2048
                pstart = ch * 512 + half * 256
                _gate_group(
                    nc, psum, gg, gf, W, 256,
                    lc[:, coff : coff + 2048], lh[:, coff : coff + 2048],
                    x15[:, L5_OFF + pstart : L5_OFF + pstart + 256],
                    hs5[:, pstart : pstart + 256],
                    fc5[:, pstart : pstart + 256],
                )
                if half == 1:
                    leaf_states.pop(ch)

            def l5_gates_part(ch, half):
                # half-chunk variant (256 parents) for the endgame pipeline
                lc, lh = leaf_states[ch]
                coff = half * 2048
                pstart = ch * 512 + half * 256
                _gate_group(
                    nc, psum, gg, gf, W, 256,
                    lc[:, coff : coff + 2048], lh[:, coff : coff + 2048],
                    x15[:, L5_OFF + pstart : L5_OFF + pstart + 256],
                    hs5[:, pstart : pstart + 256],
                    fc5[:, pstart : pstart + 256],
                )
                if half == 1:
                    leaf_states.pop(ch)

            # ---- Main loop. Engine queues are strict FIFO, so every
            # inserted stage (L5 tops, L4 gates/tops) is scheduled >=1 chunk
            # after its producers — inserted ops never head-of-line-block
            # the leaf ACT stream.
            for ch in range(N_CHUNKS):
                leaf_chunk(ch)
                if ch == 5:
                    l5top_p2s(0, 1024)
                elif ch == 6:
                    l5top_p2s(1024, 1024)
                elif ch == 7:
                    l5top_p2s(2048, 1024)
                if ch >= 1:
                    l5_gates(ch - 1)
                leaf_finish(ch)
                if ch == 4:
                    l5top_p1s(0, 1024)
                elif ch == 5:
                    l5top_p1s(1024, 1024)
                elif ch == 6:
                    l5top_p1s(2048, 1024)
                    l4gate_r(0, 128)
                elif ch == 7:
                    l4gate_r(128, 128)
                    ship4(0, 256)

            # ---- tail: only the chunk-7-dependent chain remains; ready
            # work is interleaved between the dependent stages so each has
            # slack when the ACT queue (strict FIFO) reaches it.
            l5_gates_part(7, 0)        # parents 3584:3840
            l5top_p1s(3072, 512)       # gates(6) finished a stage ago
            l4gate_r(256, 128)         # children 2048:3072 (ready)
            l5_gates_part(7, 1)        # parents 3840:4096
            l5top_p2s(3072, 512)
            l5top_p1s(3584, 256)
            l4gate_r(384, 64)          # children 3072:3584
            l5top_p2s(3584, 256)
            l5top_p1s(3840, 256)
            ship4(256, 128)
            l5top_p2s(3840, 256)
            l4gate_r(448, 64)          # children 3584:4096
            ship4(384, 128)

    nc.compile()
    return nc


_NC_CACHE = None


def _get_nc():
    global _NC_CACHE
    if _NC_CACHE is None:
        _NC_CACHE = _build_subtree_kernel()
    return _NC_CACHE


def _sigmoid(x):
    return 1.0 / (1.0 + np.exp(-x))


def kernel(
    x, W_ioux, b_ioux, W_iouh, b_iouh, W_fx, b_fx, W_fh, b_fh, branch, depth
):
    global LAST_RESULTS
    assert int(branch) == BRANCH and int(depth) == DEPTH

    x = np.asarray(x, np.float32)
    W_ioux = np.asarray(W_ioux, np.float32)
    b_ioux = np.asarray(b_ioux, np.float32)
    W_iouh = np.asarray(W_iouh, np.float32)
    b_iouh = np.asarray(b_iouh, np.float32)
    W_fx = np.asarray(W_fx, np.float32)
    b_fx = np.asarray(b_fx, np.float32)
    W_fh = np.asarray(W_fh, np.float32)
    b_fh = np.asarray(b_fh, np.float32)

    wioux = np.ascontiguousarray(W_ioux.T.astype(np.float16))
    wiouh = np.ascontiguousarray(W_iouh.T.astype(np.float16))
    wfx = np.ascontiguousarray(W_fx.T.astype(np.float16))
    wfh = np.ascontiguousarray(W_fh.T.astype(np.float16))
    biou_full = b_ioux + b_iouh
    biou = np.ascontiguousarray(biou_full.reshape(3, 128).T.astype(np.float32))
    biourow = np.ascontiguousarray(biou_full.reshape(1, 384).astype(np.float16))
    bf = np.ascontiguousarray((b_fx + b_fh).reshape(128, 1).astype(np.float32))

    off = lambda l: (BRANCH**l - 1) // (BRANCH - 1)
    in_maps = []
    for c in range(BRANCH):
        parts = []
        for l in range(1, DEPTH + 1):
            sz = BRANCH ** (l - 1)
            parts.append(x[off(l) + c * sz : off(l) + (c + 1) * sz])
        xs_c = np.ascontiguousarray(
            np.concatenate(parts, axis=0).T.astype(np.float16)
        )
        in_maps.append(
            {
                "xs": xs_c,
                "wioux": wioux,
                "wiouh": wiouh,
                "wfx": wfx,
                "wfh": wfh,
                "biou": biou,
                "biourow": biourow,
                "bf": bf,
            }
        )

    nc = _get_nc()
    trace = os.environ.get("TREELSTM_TRACE") == "1"
    res = run_bass_kernel_spmd(nc, in_maps, core_ids=list(range(8)), trace=trace)
    LAST_RESULTS = res

    # L4 partials from the device: [8, 512, 128] (node-major)
    hs4 = np.stack(
        [res.results[s]["out"][:, 0:512].T.astype(np.float32) for s in range(8)]
    )
    fc4 = np.stack(
        [res.results[s]["out"][:, 512:1024].T.astype(np.float32) for s in range(8)]
    )

    # L4 iou + cell update in fp32 on the host (the forget gates and child
    # sums for L4 ran on the device)
    xs_4 = np.stack(
        [x[off(4) + s * 512 : off(4) + (s + 1) * 512] for s in range(8)]
    )  # [8, 512, 128]
    iou = xs_4 @ W_ioux.T + b_ioux + hs4 @ W_iouh.T + b_iouh
    i, o, u = iou[..., 0:128], iou[..., 128:256], iou[..., 256:384]
    c_prev = _sigmoid(i) * np.tanh(u) + fc4
    h_prev = _sigmoid(o) * np.tanh(c_prev)

    # L3/L2/L1 of each subtree on the host (levels of 64/8/1 nodes), matching
    # the reference recursion exactly in fp32.
    for l in (3, 2, 1):
        n = BRANCH ** (l - 1)
        c_ch = c_prev.reshape(8, n, BRANCH, MEM)
        h_ch = h_prev.reshape(8, n, BRANCH, MEM)
        xs_l = np.stack(
            [x[off(l) + s * n : off(l) + (s + 1) * n] for s in range(8)]
        )  # [8, n, 128]
        h_sum = h_ch.sum(axis=2)
        f = _sigmoid(
            np.einsum("snbm,km->snbk", h_ch, W_fh)
            + b_fh
            + (xs_l @ W_fx.T + b_fx)[:, :, None, :]
        )
        fc_sum = (f * c_ch).sum(axis=2)
        iou = xs_l @ W_ioux.T + b_ioux + h_sum @ W_iouh.T + b_iouh
        i, o, u = iou[..., 0:128], iou[..., 128:256], iou[..., 256:384]
        c_prev = _sigmoid(i) * np.tanh(u) + fc_sum
        h_prev = _sigmoid(o) * np.tanh(c_prev)

    c_ch = c_prev[:, 0, :]  # [8, 128] — the root's children
    h_ch = h_prev[:, 0, :]

    # Root node on host (fp32), matching reference node_forward.
    x0 = x[0:1]  # [1,128]
    h_sum = h_ch.sum(axis=0, keepdims=True)  # [1,128]
    f = _sigmoid(h_ch @ W_fh.T + b_fh + (x0 @ W_fx.T + b_fx))  # [8,128]
    fc_sum = (f * c_ch).sum(axis=0, keepdims=True)  # [1,128]
    iou = x0 @ W_ioux.T + b_ioux + h_sum @ W_iouh.T + b_iouh  # [1,384]
    i, o, u = iou[:, 0:128], iou[:, 128:256], iou[:, 256:384]
    c_root = _sigmoid(i) * np.tanh(u) + fc_sum
    h_root = _sigmoid(o) * np.tanh(c_root)
    return (c_root.astype(np.float32), h_root.astype(np.float32))
